# revision 3
# baseline (speedup 1.0000x reference)
"""CTPN loss kernel for Trainium2 (Bass/Tile), data-parallel over 8 NeuronCores.

Strategy: the loss only touches 64 positive + 64 negative anchor locations of
the (1, 512, 1024, 50) score map. We shard the image rows (H=512) across the
8 cores (64 rows each, 13.1MB per core). Each core receives all 128 anchor
indices translated into its local row coordinates; out-of-shard anchors are
clamped and masked to zero on-device. The core then:
  1. indirect-DMA-gathers the (<=128) anchor rows [128, 50] from its HBM shard
  2. extracts the per-anchor channels (2z, 2z+1, 40+z, 20+2z, 21+2z) with an
     iota/one-hot select on the vector engine
  3. computes cross-entropy (softplus), SmoothL1 terms, masks, and reduces
     across partitions with a ones-vector matmul
  4. emits its partial scalar loss: ce_sum/128 + (lv_sum + lo_sum)/n_o
The host sums the 8 partial scalars (the data-parallel all-reduce).
"""

import numpy as np

import concourse.bacc as bacc
import concourse.bass as bass
import concourse.mybir as mybir
import concourse.tile as tile
from concourse.bass_utils import run_bass_kernel_spmd

# Problem shape (hardcoded per the harness contract)
H, W, C, K = 512, 1024, 50, 10
NP, NN = 64, 64
NCORES = 8
HS = H // NCORES          # 64 rows per core
ROWS = HS * W             # 65536 gatherable pixel-rows per core

f32 = mybir.dt.float32
i32 = mybir.dt.int32
u8 = mybir.dt.uint8

# Set by test harness to capture profiling info
TRACE = False
LAST_RESULT = None

_NC_CACHE = None


def _build_nc():
    nc = bacc.Bacc("TRN2", target_bir_lowering=False, debug=False)

    xs = nc.dram_tensor("xs", [ROWS, C], f32, kind="ExternalInput")
    # idx columns: [pos_y_local, pos_x, pos_z, neg_y_local, neg_x, neg_z]
    idx = nc.dram_tensor("idx", [NP, 6], i32, kind="ExternalInput")
    # tgt columns: [v_target0, v_target1, o_target]
    tgt = nc.dram_tensor("tgt", [NP, 3], f32, kind="ExternalInput")
    om = nc.dram_tensor("om", [NP, 1], u8, kind="ExternalInput")
    out = nc.dram_tensor("out", [1, 1], f32, kind="ExternalOutput")

    with tile.TileContext(nc) as tc:
        with (
            tc.tile_pool(name="sb", bufs=1) as pool,
            tc.tile_pool(name="ps", bufs=1, space="PSUM") as psum,
        ):
            # ---- load small inputs ----
            I = pool.tile([128, 3], i32)
            nc.sync.dma_start(I[0:64, :], idx[:, 0:3])   # pos anchors -> partitions 0-63
            nc.sync.dma_start(I[64:128, :], idx[:, 3:6])  # neg anchors -> partitions 64-127

            TGT = pool.tile([128, 3], f32)
            nc.vector.memset(TGT[:], 0.0)
            nc.sync.dma_start(TGT[0:64, :], tgt[:, :])

            OM8 = pool.tile([128, 1], u8)
            nc.vector.memset(OM8[:], 0)
            nc.sync.dma_start(OM8[0:64, :], om[:, :])
            OMf = pool.tile([128, 1], f32)
            nc.vector.tensor_copy(OMf[:], OM8[:])

            yv = I[:, 0:1]
            xv = I[:, 1:2]
            zv = I[:, 2:3]

            # ---- validity + clamped flat index ----
            vge = pool.tile([128, 1], f32)
            nc.vector.tensor_scalar(vge[:], yv, 0, None, op0=mybir.AluOpType.is_ge)
            vlt = pool.tile([128, 1], f32)
            nc.vector.tensor_scalar(vlt[:], yv, HS, None, op0=mybir.AluOpType.is_lt)
            validf = pool.tile([128, 1], f32)
            nc.vector.tensor_tensor(validf[:], vge[:], vlt[:], op=mybir.AluOpType.mult)

            yc = pool.tile([128, 1], i32)
            nc.vector.tensor_scalar(
                yc[:], yv, 0, HS - 1,
                op0=mybir.AluOpType.max, op1=mybir.AluOpType.min,
            )
            flat = pool.tile([128, 1], i32)
            nc.vector.tensor_scalar(flat[:], yc[:], W, None, op0=mybir.AluOpType.mult)
            nc.vector.tensor_tensor(flat[:], flat[:], xv, op=mybir.AluOpType.add)

            # ---- gather the 128 anchor rows from HBM ----
            G = pool.tile([128, C], f32)
            nc.gpsimd.indirect_dma_start(
                out=G[:],
                out_offset=None,
                in_=xs[:],
                in_offset=bass.IndirectOffsetOnAxis(ap=flat[:, :1], axis=0),
            )

            # ---- per-anchor channel extraction via one-hot ----
            zf = pool.tile([128, 1], f32)
            nc.vector.tensor_copy(zf[:], zv)

            # target channels, order: [v0, v1, o, cls0, cls1]
            T = pool.tile([128, 5], f32)
            nc.vector.tensor_scalar(T[:, 0:1], zf[:], 2.0, None, op0=mybir.AluOpType.mult)
            nc.vector.tensor_scalar(T[:, 1:2], zf[:], 2.0, 1.0,
                                    op0=mybir.AluOpType.mult, op1=mybir.AluOpType.add)
            nc.vector.tensor_scalar(T[:, 2:3], zf[:], 1.0, 4.0 * K,
                                    op0=mybir.AluOpType.mult, op1=mybir.AluOpType.add)
            nc.vector.tensor_scalar(T[:, 3:4], zf[:], 2.0, 2.0 * K,
                                    op0=mybir.AluOpType.mult, op1=mybir.AluOpType.add)
            nc.vector.tensor_scalar(T[:, 4:5], zf[:], 2.0, 2.0 * K + 1.0,
                                    op0=mybir.AluOpType.mult, op1=mybir.AluOpType.add)

            IO = pool.tile([128, 5 * C], f32)
            nc.gpsimd.iota(IO[:], pattern=[[0, 5], [1, C]], base=0,
                           channel_multiplier=0,
                           allow_small_or_imprecise_dtypes=True)

            IO3 = IO[:].rearrange("p (b c) -> p b c", c=C)
            T3 = T[:, :, None].to_broadcast([128, 5, C])
            MSK = pool.tile([128, 5 * C], f32)
            MSK3 = MSK[:].rearrange("p (b c) -> p b c", c=C)
            nc.vector.tensor_tensor(MSK3, IO3, T3, op=mybir.AluOpType.is_equal)

            G3 = G[:, None, :].to_broadcast([128, 5, C])
            SEL = pool.tile([128, 5 * C], f32)
            SEL3 = SEL[:].rearrange("p (b c) -> p b c", c=C)
            nc.vector.tensor_tensor(SEL3, MSK3, G3, op=mybir.AluOpType.mult)

            E = pool.tile([128, 5], f32)
            nc.vector.reduce_sum(E[:, :, None], SEL3, axis=mybir.AxisListType.X)

            # ---- is_pos / sign per partition ----
            IP = pool.tile([128, 1], i32)
            nc.gpsimd.iota(IP[:], pattern=[[0, 1]], base=0, channel_multiplier=1)
            ispos = pool.tile([128, 1], f32)
            nc.vector.tensor_scalar(ispos[:], IP[:], NP, None, op0=mybir.AluOpType.is_lt)
            signv = pool.tile([128, 1], f32)
            nc.vector.tensor_scalar(signv[:], ispos[:], 2.0, -1.0,
                                    op0=mybir.AluOpType.mult, op1=mybir.AluOpType.add)

            # ---- classification CE: softplus(sign * (cls0 - cls1)) ----
            D = pool.tile([128, 4], f32)
            dcls = pool.tile([128, 1], f32)
            nc.vector.tensor_tensor(dcls[:], E[:, 3:4], E[:, 4:5],
                                    op=mybir.AluOpType.subtract)
            nc.vector.tensor_tensor(dcls[:], dcls[:], signv[:], op=mybir.AluOpType.mult)
            # softplus(d) = max(d, 0) + ln(1 + exp(-|d|))  (no softplus ACT table)
            negd = pool.tile([128, 1], f32)
            nc.vector.tensor_scalar(negd[:], dcls[:], -1.0, None, op0=mybir.AluOpType.mult)
            nabs = pool.tile([128, 1], f32)
            nc.vector.tensor_tensor(nabs[:], dcls[:], negd[:], op=mybir.AluOpType.min)
            ex = pool.tile([128, 1], f32)
            nc.scalar.activation(ex[:], nabs[:], mybir.ActivationFunctionType.Exp)
            l1p = pool.tile([128, 1], f32)
            nc.scalar.activation(l1p[:], ex[:], mybir.ActivationFunctionType.Ln, bias=1.0)
            ce = pool.tile([128, 1], f32)
            nc.vector.tensor_scalar(ce[:], dcls[:], 0.0, None, op0=mybir.AluOpType.max)
            nc.vector.tensor_tensor(ce[:], ce[:], l1p[:], op=mybir.AluOpType.add)
            nc.vector.tensor_tensor(D[:, 0:1], ce[:], validf[:], op=mybir.AluOpType.mult)

            # ---- smooth-L1 on [v0-v0t, v1-v1t, o-ot] ----
            dreg = pool.tile([128, 3], f32)
            nc.vector.tensor_tensor(dreg[:], E[:, 0:3], TGT[:, :], op=mybir.AluOpType.subtract)
            av = pool.tile([128, 3], f32)
            nc.scalar.activation(av[:], dreg[:], mybir.ActivationFunctionType.Abs)
            mn = pool.tile([128, 3], f32)
            nc.vector.tensor_scalar(mn[:], av[:], 1.0, None, op0=mybir.AluOpType.min)
            mx = pool.tile([128, 3], f32)
            nc.vector.tensor_scalar(mx[:], av[:], 1.0, None, op0=mybir.AluOpType.max)
            sl = pool.tile([128, 3], f32)
            nc.vector.tensor_tensor(sl[:], mn[:], mn[:], op=mybir.AluOpType.mult)
            nc.vector.tensor_scalar(sl[:], sl[:], 0.5, None, op0=mybir.AluOpType.mult)
            nc.vector.tensor_tensor(sl[:], sl[:], mx[:], op=mybir.AluOpType.add)
            nc.vector.tensor_scalar(sl[:], sl[:], 1.0, None, op0=mybir.AluOpType.subtract)

            pv = pool.tile([128, 1], f32)
            nc.vector.tensor_tensor(pv[:], ispos[:], validf[:], op=mybir.AluOpType.mult)

            # lv contribution: mean of the two v-terms, pos+valid anchors only
            lv = pool.tile([128, 1], f32)
            nc.vector.tensor_tensor(lv[:], sl[:, 0:1], sl[:, 1:2], op=mybir.AluOpType.add)
            nc.vector.tensor_scalar(lv[:], lv[:], 0.5, None, op0=mybir.AluOpType.mult)
            nc.vector.tensor_tensor(D[:, 1:2], lv[:], pv[:], op=mybir.AluOpType.mult)

            # lo contribution: side-refinement, masked by o_mask, pos+valid only
            lo = pool.tile([128, 1], f32)
            nc.vector.tensor_tensor(lo[:], sl[:, 2:3], OMf[:], op=mybir.AluOpType.mult)
            nc.vector.tensor_tensor(D[:, 2:3], lo[:], pv[:], op=mybir.AluOpType.mult)

            # n_o column (o_mask is replicated on every core; zero for p>=64)
            nc.vector.tensor_copy(D[:, 3:4], OMf[:])

            # ---- partition reduction via ones-vector matmul ----
            ones = pool.tile([128, 1], f32)
            nc.vector.memset(ones[:], 1.0)
            P4 = psum.tile([1, 4], f32)
            nc.tensor.matmul(out=P4[:], lhsT=ones[:], rhs=D[:], start=True, stop=True)
            F = pool.tile([1, 4], f32)
            nc.vector.tensor_copy(F[:], P4[:])

            # partial_loss = S_ce/128 + (S_lv + S_lo)/n_o
            t = pool.tile([1, 1], f32)
            nc.vector.tensor_tensor(t[:], F[:, 1:2], F[:, 2:3], op=mybir.AluOpType.add)
            rcp = pool.tile([1, 1], f32)
            nc.vector.reciprocal(rcp[:], F[:, 3:4])
            r = pool.tile([1, 1], f32)
            nc.vector.tensor_tensor(r[:], t[:], rcp[:], op=mybir.AluOpType.mult)
            res = pool.tile([1, 1], f32)
            nc.vector.tensor_scalar(res[:], F[:, 0:1], 1.0 / (NP + NN), None,
                                    op0=mybir.AluOpType.mult)
            nc.vector.tensor_tensor(res[:], res[:], r[:], op=mybir.AluOpType.add)

            nc.sync.dma_start(out[:], res[:])

    nc.compile()
    return nc


def _get_nc():
    global _NC_CACHE
    if _NC_CACHE is None:
        _NC_CACHE = _build_nc()
    return _NC_CACHE


def make_in_maps(x, v_targets, o_targets, pos_y, pos_x, pos_z,
                 neg_y, neg_x, neg_z, o_mask):
    """Shard the full inputs into per-core input maps (host-side slicing only)."""
    xr = np.ascontiguousarray(x).reshape(H * W, C)
    tgt = np.concatenate(
        [v_targets.astype(np.float32), o_targets.astype(np.float32)[:, None]], axis=1
    )
    omu = np.ascontiguousarray(o_mask).view(np.uint8).reshape(NP, 1)
    in_maps = []
    for i in range(NCORES):
        idx = np.stack(
            [
                pos_y.astype(np.int32) - HS * i, pos_x.astype(np.int32),
                pos_z.astype(np.int32),
                neg_y.astype(np.int32) - HS * i, neg_x.astype(np.int32),
                neg_z.astype(np.int32),
            ],
            axis=1,
        ).astype(np.int32)
        in_maps.append(
            {
                "xs": xr[HS * W * i: HS * W * (i + 1)],
                "idx": idx,
                "tgt": tgt,
                "om": omu,
            }
        )
    return in_maps


def kernel(**inputs):
    global LAST_RESULT
    nc = _get_nc()
    in_maps = make_in_maps(**inputs)
    res = run_bass_kernel_spmd(nc, in_maps, core_ids=list(range(NCORES)), trace=TRACE)
    LAST_RESULT = res
    total = np.float64(0.0)
    for core_out in res.results:
        total += np.float64(core_out["out"][0, 0])
    return np.array(np.float32(total))


# revision 8
# speedup vs baseline: 1.1266x; 1.1266x over previous
"""CTPN loss kernel for Trainium2 (Bass/Tile), data-parallel over 8 NeuronCores.

Strategy: the loss only touches 64 positive + 64 negative anchor locations of
the (1, 512, 1024, 50) score map. We shard the image rows (H=512) across the
8 cores (64 rows each, 13.1MB per core). Each core receives all 128 anchor
indices translated into its local row coordinates; out-of-shard anchors are
clamped and masked to zero on-device. The core then:
  1. indirect-DMA-gathers the anchor rows [128, 50] from its HBM shard
  2. extracts the per-anchor channels (2z, 2z+1, 40+z, 20+2z, 21+2z) with an
     iota/one-hot select on the vector engine
  3. computes cross-entropy (ln(1+exp(+-d)) on the ACT engine), SmoothL1
     terms, masks, and reduces across partitions with a ones-vector matmul
  4. emits its partial scalar loss: ce_sum/128 + (lv_sum + lo_sum)/n_o
The host sums the 8 partial scalars (the data-parallel all-reduce).
"""

import types

import numpy as np

import bass_rust as _bass_rust
import concourse.bacc as bacc
import concourse.bass as bass
import concourse.mybir as mybir
import concourse.tile as tile
from concourse.bass_utils import run_bass_kernel_spmd
from concourse.hw_specs import get_activation_tables

# Problem shape (hardcoded per the harness contract)
H, W, C, K = 512, 1024, 50, 10
NP, NN = 64, 64
NCORES = 8
HS = H // NCORES          # 64 rows per core
ROWS = HS * W             # 65536 gatherable pixel-rows per core

f32 = mybir.dt.float32
i32 = mybir.dt.int32
u32 = mybir.dt.uint32
u8 = mybir.dt.uint8
Alu = mybir.AluOpType
Act = mybir.ActivationFunctionType

# Set by test harness to capture profiling info
TRACE = False
LAST_RESULT = None

_NC_CACHE = None


def _patched_insert_act_table_loads(self):
    """Restrict the ACT-table chooser to natural_log_exp_and_others so Exp and
    Ln resolve to ONE table (the default greedy pass picks two different
    tables, costing an extra mid-kernel ACT_TABLE_LOAD + drain)."""
    has_activation = any(
        isinstance(i, mybir.InstActivation)
        for b in self.main_func.blocks
        for i in b.instructions
    )
    if not has_activation:
        return
    tables = [
        (name, funcs if name == "natural_log_exp_and_others" else set())
        for name, funcs in get_activation_tables(self.m.arch).items()
    ]
    _bass_rust.insert_act_table_loads(self, tables)


def _build_nc():
    nc = bacc.Bacc("TRN2", target_bir_lowering=False, debug=False)
    nc.insert_act_table_loads = types.MethodType(_patched_insert_act_table_loads, nc)

    xs = nc.dram_tensor("xs", [ROWS, C], f32, kind="ExternalInput")
    # idx columns: [pos_y_local, pos_x, pos_z, neg_y_local, neg_x, neg_z]
    idx = nc.dram_tensor("idx", [NP, 6], i32, kind="ExternalInput")
    # tgt columns: [v_target0, v_target1, o_target]
    tgt = nc.dram_tensor("tgt", [NP, 3], f32, kind="ExternalInput")
    om = nc.dram_tensor("om", [NP, 1], u8, kind="ExternalInput")
    out = nc.dram_tensor("out", [1, 1], f32, kind="ExternalOutput")

    with tile.TileContext(nc) as tc:
        with (
            tc.tile_pool(name="sb", bufs=1) as pool,
            tc.tile_pool(name="ps", bufs=1, space="PSUM") as psum,
        ):
            # ======== critical path: idx DMA -> flat index -> gather ========
            # one DMA for both halves: partitions 0-63 <- idx[:,0:3] (pos),
            # partitions 64-127 <- idx[:,3:6] (neg)
            I = pool.tile([128, 3], i32)
            nc.sync.dma_start(I[:], idx[:].rearrange("r (h c) -> h r c", h=2))

            yv = I[:, 0:1]
            xv = I[:, 1:2]
            zv = I[:, 2:3]

            yc = pool.tile([128, 1], i32)
            nc.vector.tensor_scalar(yc[:], yv, 0, HS - 1, op0=Alu.max, op1=Alu.min)
            flat = pool.tile([128, 1], i32)
            nc.vector.tensor_scalar(flat[:], yc[:], W, None, op0=Alu.mult)
            nc.vector.tensor_tensor(flat[:], flat[:], xv, op=Alu.add)

            G = pool.tile([128, C], f32)
            nc.gpsimd.indirect_dma_start(
                out=G[:],
                out_offset=None,
                in_=xs[:],
                in_offset=bass.IndirectOffsetOnAxis(ap=flat[:, :1], axis=0),
            )

            # ======== off-path prep (overlaps the DMA flights) ========
            # remaining small inputs
            TGT = pool.tile([128, 3], f32)
            nc.vector.memset(TGT[64:128, :], 0.0)
            nc.sync.dma_start(TGT[0:64, :], tgt[:, :])
            OM8 = pool.tile([128, 1], u8)
            nc.vector.memset(OM8[64:128, :], 0)
            nc.sync.dma_start(OM8[0:64, :], om[:, :])

            D = pool.tile([128, 4], f32)
            nc.vector.tensor_copy(D[:, 3:4], OM8[:])  # n_o column (u8 -> f32)

            # per-partition constants
            psign = pool.tile([128, 1], f32)   # +1 pos rows, -1 neg rows
            nc.vector.memset(psign[0:64, :], 1.0)
            nc.vector.memset(psign[64:128, :], -1.0)
            ispos = pool.tile([128, 1], f32)
            nc.vector.memset(ispos[0:64, :], 1.0)
            nc.vector.memset(ispos[64:128, :], 0.0)
            ones = pool.tile([128, 1], f32)
            nc.vector.memset(ones[:], 1.0)

            # validity: 0 <= y_local < HS  (single unsigned compare)
            vf = pool.tile([128, 1], f32)
            nc.vector.tensor_scalar(vf[:], yv.bitcast(u32), HS, None, op0=Alu.is_lt)
            vf128 = pool.tile([128, 1], f32)
            nc.vector.tensor_scalar(vf128[:], vf[:], 1.0 / (NP + NN), None, op0=Alu.mult)
            pv = pool.tile([128, 1], f32)
            nc.vector.tensor_tensor(pv[:], ispos[:], vf[:], op=Alu.mult)
            pvh = pool.tile([128, 1], f32)
            nc.vector.tensor_scalar(pvh[:], pv[:], 0.5, None, op0=Alu.mult)
            ompv = pool.tile([128, 1], f32)
            nc.vector.tensor_tensor(ompv[:], D[:, 3:4], pv[:], op=Alu.mult)

            # channel-target one-hot prep, order: [v0, v1, o, cls0, cls1]
            zf = pool.tile([128, 1], f32)
            nc.vector.tensor_copy(zf[:], zv)
            T = pool.tile([128, 5], f32)
            nc.vector.tensor_scalar(T[:, 0:1], zf[:], 2.0, None, op0=Alu.mult)
            nc.vector.tensor_scalar(T[:, 1:2], zf[:], 2.0, 1.0, op0=Alu.mult, op1=Alu.add)
            nc.vector.tensor_scalar(T[:, 2:3], zf[:], 1.0, 4.0 * K, op0=Alu.mult, op1=Alu.add)
            nc.vector.tensor_scalar(T[:, 3:4], zf[:], 2.0, 2.0 * K, op0=Alu.mult, op1=Alu.add)
            nc.vector.tensor_scalar(T[:, 4:5], zf[:], 2.0, 2.0 * K + 1.0, op0=Alu.mult, op1=Alu.add)

            IO = pool.tile([128, 5 * C], f32)
            nc.gpsimd.iota(IO[:], pattern=[[0, 5], [1, C]], base=0,
                           channel_multiplier=0,
                           allow_small_or_imprecise_dtypes=True)
            IO3 = IO[:].rearrange("p (b c) -> p b c", c=C)
            T3 = T[:, :, None].to_broadcast([128, 5, C])
            MSK = pool.tile([128, 5 * C], f32)
            MSK3 = MSK[:].rearrange("p (b c) -> p b c", c=C)
            nc.vector.tensor_tensor(MSK3, IO3, T3, op=Alu.is_equal)

            # ======== post-gather: extract channels ========
            G3 = G[:, None, :].to_broadcast([128, 5, C])
            SEL = pool.tile([128, 5 * C], f32)
            SEL3 = SEL[:].rearrange("p (b c) -> p b c", c=C)
            nc.vector.tensor_tensor(SEL3, MSK3, G3, op=Alu.mult)
            E = pool.tile([128, 5], f32)
            nc.vector.reduce_sum(E[:, :, None], SEL3, axis=mybir.AxisListType.X)

            # ======== classification CE = ln(1 + exp(psign*(cls0-cls1))) ========
            dcls = pool.tile([128, 1], f32)
            nc.vector.tensor_tensor(dcls[:], E[:, 3:4], E[:, 4:5], op=Alu.subtract)
            ex = pool.tile([128, 1], f32)
            nc.scalar.activation(ex[:], dcls[:], Act.Exp, scale=psign[:])
            ce = pool.tile([128, 1], f32)
            nc.scalar.activation(ce[:], ex[:], Act.Ln, bias=1.0)
            nc.vector.tensor_tensor(D[:, 0:1], ce[:], vf128[:], op=Alu.mult)

            # ======== smooth-L1 on [v0-v0t, v1-v1t, o-ot] ========
            # sl1(d) = 0.5*min(|d|,1)^2 + max(|d|,1) - 1
            dreg = pool.tile([128, 3], f32)
            nc.vector.tensor_tensor(dreg[:], E[:, 0:3], TGT[:, :], op=Alu.subtract)
            ngd = pool.tile([128, 3], f32)
            nc.vector.tensor_scalar(ngd[:], dreg[:], -1.0, None, op0=Alu.mult)
            av = pool.tile([128, 3], f32)
            nc.vector.tensor_tensor(av[:], dreg[:], ngd[:], op=Alu.max)
            mn = pool.tile([128, 3], f32)
            nc.vector.tensor_scalar(mn[:], av[:], 1.0, None, op0=Alu.min)
            sq = pool.tile([128, 3], f32)
            nc.vector.tensor_tensor(sq[:], mn[:], mn[:], op=Alu.mult)
            sqh = pool.tile([128, 3], f32)
            nc.vector.tensor_scalar(sqh[:], sq[:], 0.5, None, op0=Alu.mult)
            mx1 = pool.tile([128, 3], f32)
            nc.vector.tensor_scalar(mx1[:], av[:], 1.0, 1.0, op0=Alu.max, op1=Alu.subtract)
            sl = pool.tile([128, 3], f32)
            nc.vector.tensor_tensor(sl[:], sqh[:], mx1[:], op=Alu.add)

            lvs = pool.tile([128, 1], f32)
            nc.vector.tensor_tensor(lvs[:], sl[:, 0:1], sl[:, 1:2], op=Alu.add)
            nc.vector.tensor_tensor(D[:, 1:2], lvs[:], pvh[:], op=Alu.mult)
            nc.vector.tensor_tensor(D[:, 2:3], sl[:, 2:3], ompv[:], op=Alu.mult)

            # ======== partition reduction + combine ========
            P4 = psum.tile([1, 4], f32)
            nc.tensor.matmul(out=P4[:], lhsT=ones[:], rhs=D[:], start=True, stop=True)
            F = pool.tile([1, 4], f32)
            nc.vector.tensor_copy(F[:], P4[:])
            t = pool.tile([1, 1], f32)
            nc.vector.tensor_tensor(t[:], F[:, 1:2], F[:, 2:3], op=Alu.add)
            rcp = pool.tile([1, 1], f32)
            nc.vector.reciprocal(rcp[:], F[:, 3:4])
            r = pool.tile([1, 1], f32)
            nc.vector.tensor_tensor(r[:], t[:], rcp[:], op=Alu.mult)
            res = pool.tile([1, 1], f32)
            nc.vector.tensor_tensor(res[:], F[:, 0:1], r[:], op=Alu.add)

            nc.sync.dma_start(out[:], res[:])

    nc.compile()
    return nc


def _get_nc():
    global _NC_CACHE
    if _NC_CACHE is None:
        _NC_CACHE = _build_nc()
    return _NC_CACHE


def make_in_maps(x, v_targets, o_targets, pos_y, pos_x, pos_z,
                 neg_y, neg_x, neg_z, o_mask):
    """Shard the full inputs into per-core input maps (host-side slicing only)."""
    xr = np.ascontiguousarray(x).reshape(H * W, C)
    tgt = np.concatenate(
        [v_targets.astype(np.float32), o_targets.astype(np.float32)[:, None]], axis=1
    )
    omu = np.ascontiguousarray(o_mask).view(np.uint8).reshape(NP, 1)
    in_maps = []
    for i in range(NCORES):
        idx = np.stack(
            [
                pos_y.astype(np.int32) - HS * i, pos_x.astype(np.int32),
                pos_z.astype(np.int32),
                neg_y.astype(np.int32) - HS * i, neg_x.astype(np.int32),
                neg_z.astype(np.int32),
            ],
            axis=1,
        ).astype(np.int32)
        in_maps.append(
            {
                "xs": xr[HS * W * i: HS * W * (i + 1)],
                "idx": idx,
                "tgt": tgt,
                "om": omu,
            }
        )
    return in_maps


def kernel(**inputs):
    global LAST_RESULT
    nc = _get_nc()
    in_maps = make_in_maps(**inputs)
    res = run_bass_kernel_spmd(nc, in_maps, core_ids=list(range(NCORES)), trace=TRACE)
    LAST_RESULT = res
    total = np.float64(0.0)
    for core_out in res.results:
        total += np.float64(core_out["out"][0, 0])
    return np.array(np.float32(total))


# revision 11
# speedup vs baseline: 1.1305x; 1.0034x over previous
"""CTPN loss kernel for Trainium2 (Bass/Tile), data-parallel over 8 NeuronCores.

Strategy: the loss only touches 64 positive + 64 negative anchor locations of
the (1, 512, 1024, 50) score map. We shard the image rows (H=512) across the
8 cores (64 rows each, 13.1MB per core). Each core receives all 128 anchor
indices translated into its local row coordinates; out-of-shard anchors are
clamped and masked to zero on-device. The core then:
  1. indirect-DMA-gathers the anchor rows [128, 50] from its HBM shard
  2. extracts the per-anchor channels (2z, 2z+1, 40+z, 20+2z, 21+2z) with an
     iota/one-hot select on the vector engine
  3. computes cross-entropy (ln(1+exp(+-d)) on the ACT engine), SmoothL1
     terms, masks, and reduces across partitions with a ones-vector matmul
  4. emits its partial scalar loss: ce_sum/128 + (lv_sum + lo_sum)/n_o
The host sums the 8 partial scalars (the data-parallel all-reduce).
"""

import types

import numpy as np

import bass_rust as _bass_rust
import concourse.bacc as bacc
import concourse.bass as bass
import concourse.mybir as mybir
import concourse.tile as tile
from concourse.bass_utils import run_bass_kernel_spmd
from concourse.tile import add_dep_helper
from concourse.hw_specs import get_activation_tables

# Problem shape (hardcoded per the harness contract)
H, W, C, K = 512, 1024, 50, 10
NP, NN = 64, 64
NCORES = 8
HS = H // NCORES          # 64 rows per core
ROWS = HS * W             # 65536 gatherable pixel-rows per core

f32 = mybir.dt.float32
i32 = mybir.dt.int32
u32 = mybir.dt.uint32
u8 = mybir.dt.uint8
Alu = mybir.AluOpType
Act = mybir.ActivationFunctionType

# Set by test harness to capture profiling info
TRACE = False
LAST_RESULT = None

_NC_CACHE = None


def _patched_insert_act_table_loads(self):
    """Restrict the ACT-table chooser to natural_log_exp_and_others so Exp and
    Ln resolve to ONE table (the default greedy pass picks two different
    tables, costing an extra mid-kernel ACT_TABLE_LOAD + drain)."""
    has_activation = any(
        isinstance(i, mybir.InstActivation)
        for b in self.main_func.blocks
        for i in b.instructions
    )
    if not has_activation:
        return
    tables = [
        (name, funcs if name == "natural_log_exp_and_others" else set())
        for name, funcs in get_activation_tables(self.m.arch).items()
    ]
    _bass_rust.insert_act_table_loads(self, tables)


def _build_nc():
    nc = bacc.Bacc("TRN2", target_bir_lowering=False, debug=False)
    nc.insert_act_table_loads = types.MethodType(_patched_insert_act_table_loads, nc)

    xs = nc.dram_tensor("xs", [ROWS, C], f32, kind="ExternalInput")
    # idx columns: [pos_y_local, pos_x, pos_z, neg_y_local, neg_x, neg_z]
    idx = nc.dram_tensor("idx", [NP, 6], i32, kind="ExternalInput")
    # tgt columns: [v_target0, v_target1, o_target]
    tgt = nc.dram_tensor("tgt", [NP, 3], f32, kind="ExternalInput")
    om = nc.dram_tensor("om", [NP, 1], u8, kind="ExternalInput")
    out = nc.dram_tensor("out", [1, 1], f32, kind="ExternalOutput")

    with tile.TileContext(nc) as tc:
        with (
            tc.tile_pool(name="sb", bufs=1) as pool,
            tc.tile_pool(name="ps", bufs=1, space="PSUM") as psum,
        ):
            # ======== critical path: idx DMA -> flat index -> gather ========
            # one DMA for both halves: partitions 0-63 <- idx[:,0:3] (pos),
            # partitions 64-127 <- idx[:,3:6] (neg)
            I = pool.tile([128, 3], i32)
            nc.sync.dma_start(I[:], idx[:].rearrange("r (h c) -> h r c", h=2))

            yv = I[:, 0:1]
            xv = I[:, 1:2]
            zv = I[:, 2:3]

            yc = pool.tile([128, 1], i32)
            nc.vector.tensor_scalar(yc[:], yv, 0, HS - 1, op0=Alu.max, op1=Alu.min)
            flat = pool.tile([128, 1], i32)
            nc.vector.tensor_scalar(flat[:], yc[:], W, None, op0=Alu.mult)
            nc.vector.tensor_tensor(flat[:], flat[:], xv, op=Alu.add)

            G = pool.tile([128, C], f32)
            gather = nc.gpsimd.indirect_dma_start(
                out=G[:],
                out_offset=None,
                in_=xs[:],
                in_offset=bass.IndirectOffsetOnAxis(ap=flat[:, :1], axis=0),
            )

            def _off_path(inst):
                # order after the gather issue so the scheduler cannot slot
                # these between the flat-index chain and the gather
                add_dep_helper(gather.ins, inst.ins, sync=False,
                               reason="keep flat->gather chain tight")
                return inst

            # ======== off-path prep (overlaps the DMA flights) ========
            # remaining small inputs
            TGT = pool.tile([128, 3], f32)
            nc.vector.memset(TGT[64:128, :], 0.0)
            nc.sync.dma_start(TGT[0:64, :], tgt[:, :])
            OM8 = pool.tile([128, 1], u8)
            nc.vector.memset(OM8[64:128, :], 0)
            nc.sync.dma_start(OM8[0:64, :], om[:, :])

            D = pool.tile([128, 4], f32)
            _off_path(nc.vector.tensor_copy(D[:, 3:4], OM8[:]))  # n_o column (u8 -> f32)

            # per-partition constants
            psign = pool.tile([128, 1], f32)   # +1 pos rows, -1 neg rows
            nc.vector.memset(psign[0:64, :], 1.0)
            nc.vector.memset(psign[64:128, :], -1.0)
            ispos = pool.tile([128, 1], f32)
            nc.vector.memset(ispos[0:64, :], 1.0)
            nc.vector.memset(ispos[64:128, :], 0.0)
            ones = pool.tile([128, 1], f32)
            nc.vector.memset(ones[:], 1.0)

            # validity: 0 <= y_local < HS  (single unsigned compare)
            vf = pool.tile([128, 1], f32)
            _off_path(nc.vector.tensor_scalar(vf[:], yv.bitcast(u32), HS, None, op0=Alu.is_lt))
            vf128 = pool.tile([128, 1], f32)
            _off_path(nc.vector.tensor_scalar(vf128[:], vf[:], 1.0 / (NP + NN), None, op0=Alu.mult))
            pv = pool.tile([128, 1], f32)
            _off_path(nc.vector.tensor_tensor(pv[:], ispos[:], vf[:], op=Alu.mult))
            pvh = pool.tile([128, 1], f32)
            _off_path(nc.vector.tensor_scalar(pvh[:], pv[:], 0.5, None, op0=Alu.mult))
            ompv = pool.tile([128, 1], f32)
            _off_path(nc.vector.tensor_tensor(ompv[:], D[:, 3:4], pv[:], op=Alu.mult))

            # channel-target one-hot prep, order: [v0, v1, o, cls0, cls1]
            zf = pool.tile([128, 1], f32)
            _off_path(nc.vector.tensor_copy(zf[:], zv))
            T = pool.tile([128, 5], f32)
            _off_path(nc.vector.tensor_scalar(T[:, 0:1], zf[:], 2.0, None, op0=Alu.mult))
            _off_path(nc.vector.tensor_scalar(T[:, 1:2], zf[:], 2.0, 1.0, op0=Alu.mult, op1=Alu.add))
            _off_path(nc.vector.tensor_scalar(T[:, 2:3], zf[:], 1.0, 4.0 * K, op0=Alu.mult, op1=Alu.add))
            _off_path(nc.vector.tensor_scalar(T[:, 3:4], zf[:], 2.0, 2.0 * K, op0=Alu.mult, op1=Alu.add))
            _off_path(nc.vector.tensor_scalar(T[:, 4:5], zf[:], 2.0, 2.0 * K + 1.0, op0=Alu.mult, op1=Alu.add))

            IO = pool.tile([128, 5 * C], f32)
            nc.gpsimd.iota(IO[:], pattern=[[0, 5], [1, C]], base=0,
                           channel_multiplier=0,
                           allow_small_or_imprecise_dtypes=True)
            IO3 = IO[:].rearrange("p (b c) -> p b c", c=C)
            T3 = T[:, :, None].to_broadcast([128, 5, C])
            MSK = pool.tile([128, 5 * C], f32)
            MSK3 = MSK[:].rearrange("p (b c) -> p b c", c=C)
            _off_path(nc.vector.tensor_tensor(MSK3, IO3, T3, op=Alu.is_equal))

            # ======== post-gather: extract channels ========
            G3 = G[:, None, :].to_broadcast([128, 5, C])
            SEL = pool.tile([128, 5 * C], f32)
            SEL3 = SEL[:].rearrange("p (b c) -> p b c", c=C)
            nc.vector.tensor_tensor(SEL3, MSK3, G3, op=Alu.mult)
            E = pool.tile([128, 5], f32)
            nc.vector.reduce_sum(E[:, :, None], SEL3, axis=mybir.AxisListType.X)

            # ======== classification CE = ln(1 + exp(psign*(cls0-cls1))) ========
            dcls = pool.tile([128, 1], f32)
            nc.vector.tensor_tensor(dcls[:], E[:, 3:4], E[:, 4:5], op=Alu.subtract)
            ex = pool.tile([128, 1], f32)
            nc.scalar.activation(ex[:], dcls[:], Act.Exp, scale=psign[:])
            ce = pool.tile([128, 1], f32)
            nc.scalar.activation(ce[:], ex[:], Act.Ln, bias=1.0)
            nc.vector.tensor_tensor(D[:, 0:1], ce[:], vf128[:], op=Alu.mult)

            # ======== smooth-L1 on [v0-v0t, v1-v1t, o-ot] ========
            # sl1(d) = 0.5*min(|d|,1)^2 + max(|d|,1) - 1
            dreg = pool.tile([128, 3], f32)
            nc.vector.tensor_tensor(dreg[:], E[:, 0:3], TGT[:, :], op=Alu.subtract)
            ngd = pool.tile([128, 3], f32)
            nc.vector.tensor_scalar(ngd[:], dreg[:], -1.0, None, op0=Alu.mult)
            av = pool.tile([128, 3], f32)
            nc.vector.tensor_tensor(av[:], dreg[:], ngd[:], op=Alu.max)
            mn = pool.tile([128, 3], f32)
            nc.vector.tensor_scalar(mn[:], av[:], 1.0, None, op0=Alu.min)
            sq = pool.tile([128, 3], f32)
            nc.vector.tensor_tensor(sq[:], mn[:], mn[:], op=Alu.mult)
            sqh = pool.tile([128, 3], f32)
            nc.vector.tensor_scalar(sqh[:], sq[:], 0.5, None, op0=Alu.mult)
            mx1 = pool.tile([128, 3], f32)
            nc.vector.tensor_scalar(mx1[:], av[:], 1.0, 1.0, op0=Alu.max, op1=Alu.subtract)
            sl = pool.tile([128, 3], f32)
            nc.vector.tensor_tensor(sl[:], sqh[:], mx1[:], op=Alu.add)

            lvs = pool.tile([128, 1], f32)
            nc.vector.tensor_tensor(lvs[:], sl[:, 0:1], sl[:, 1:2], op=Alu.add)
            nc.vector.tensor_tensor(D[:, 1:2], lvs[:], pvh[:], op=Alu.mult)
            nc.vector.tensor_tensor(D[:, 2:3], sl[:, 2:3], ompv[:], op=Alu.mult)

            # ======== partition reduction + combine ========
            P4 = psum.tile([1, 4], f32)
            nc.tensor.matmul(out=P4[:], lhsT=ones[:], rhs=D[:], start=True, stop=True)
            F = pool.tile([1, 4], f32)
            nc.vector.tensor_copy(F[:], P4[:])
            t = pool.tile([1, 1], f32)
            nc.vector.tensor_tensor(t[:], F[:, 1:2], F[:, 2:3], op=Alu.add)
            rcp = pool.tile([1, 1], f32)
            nc.vector.reciprocal(rcp[:], F[:, 3:4])
            r = pool.tile([1, 1], f32)
            nc.vector.tensor_tensor(r[:], t[:], rcp[:], op=Alu.mult)
            res = pool.tile([1, 1], f32)
            nc.vector.tensor_tensor(res[:], F[:, 0:1], r[:], op=Alu.add)

            nc.sync.dma_start(out[:], res[:])

    nc.compile()
    return nc


def _get_nc():
    global _NC_CACHE
    if _NC_CACHE is None:
        _NC_CACHE = _build_nc()
    return _NC_CACHE


def make_in_maps(x, v_targets, o_targets, pos_y, pos_x, pos_z,
                 neg_y, neg_x, neg_z, o_mask):
    """Shard the full inputs into per-core input maps (host-side slicing only)."""
    xr = np.ascontiguousarray(x).reshape(H * W, C)
    tgt = np.concatenate(
        [v_targets.astype(np.float32), o_targets.astype(np.float32)[:, None]], axis=1
    )
    omu = np.ascontiguousarray(o_mask).view(np.uint8).reshape(NP, 1)
    in_maps = []
    for i in range(NCORES):
        idx = np.stack(
            [
                pos_y.astype(np.int32) - HS * i, pos_x.astype(np.int32),
                pos_z.astype(np.int32),
                neg_y.astype(np.int32) - HS * i, neg_x.astype(np.int32),
                neg_z.astype(np.int32),
            ],
            axis=1,
        ).astype(np.int32)
        in_maps.append(
            {
                "xs": xr[HS * W * i: HS * W * (i + 1)],
                "idx": idx,
                "tgt": tgt,
                "om": omu,
            }
        )
    return in_maps


def kernel(**inputs):
    global LAST_RESULT
    nc = _get_nc()
    in_maps = make_in_maps(**inputs)
    res = run_bass_kernel_spmd(nc, in_maps, core_ids=list(range(NCORES)), trace=TRACE)
    LAST_RESULT = res
    total = np.float64(0.0)
    for core_out in res.results:
        total += np.float64(core_out["out"][0, 0])
    return np.array(np.float32(total))


# revision 13
# speedup vs baseline: 1.1963x; 1.0582x over previous
"""CTPN loss kernel for Trainium2 (Bass/Tile), data-parallel over 8 NeuronCores.

Strategy: the loss only touches 64 positive + 64 negative anchor locations of
the (1, 512, 1024, 50) score map. We shard the image rows (H=512) across the
8 cores (64 rows each, 13.1MB per core). Each core receives the 128 anchor
indices translated into its local row-major layout (clamped; out-of-shard
anchors are masked to zero on-device via the raw y coordinate). Per core:
  1. an indirect DMA gathers the anchor rows [128, 50] straight from HBM,
     reading its per-partition offsets directly from DRAM (first instruction,
     no SBUF index round-trip on the critical path)
  2. per-anchor channels (2z, 2z+1, 40+z, 20+2z, 21+2z) are extracted with an
     iota/one-hot select on the vector engine
  3. cross-entropy ln(1+exp(+-d)) runs on the ACT engine (single exp+ln
     table), SmoothL1 via 0.5*min(d^2,1) + max(|d|,1) - 1, masks folded into
     per-partition weights, partition-reduce via a ones-vector matmul
  4. each core emits its partial scalar loss: ce_sum/128 + (lv_sum+lo_sum)/n_o
The host sums the 8 partial scalars (the data-parallel all-reduce).
"""

import types

import numpy as np

import bass_rust as _bass_rust
import concourse.bacc as bacc
import concourse.bass as bass
import concourse.mybir as mybir
import concourse.tile as tile
from concourse.bass_utils import run_bass_kernel_spmd
from concourse.hw_specs import get_activation_tables

# Problem shape (hardcoded per the harness contract)
H, W, C, K = 512, 1024, 50, 10
NP, NN = 64, 64
NCORES = 8
HS = H // NCORES          # 64 rows per core
ROWS = HS * W             # 65536 gatherable pixel-rows per core

f32 = mybir.dt.float32
i32 = mybir.dt.int32
u32 = mybir.dt.uint32
u8 = mybir.dt.uint8
Alu = mybir.AluOpType
Act = mybir.ActivationFunctionType

# Set by test harness to capture profiling info
TRACE = False
LAST_RESULT = None

_NC_CACHE = None


def _patched_insert_act_table_loads(self):
    """Restrict the ACT-table chooser to natural_log_exp_and_others so Exp and
    Ln resolve to ONE table (the default greedy pass picks two different
    tables, costing an extra mid-kernel ACT_TABLE_LOAD + drain)."""
    has_activation = any(
        isinstance(i, mybir.InstActivation)
        for b in self.main_func.blocks
        for i in b.instructions
    )
    if not has_activation:
        return
    tables = [
        (name, funcs if name == "natural_log_exp_and_others" else set())
        for name, funcs in get_activation_tables(self.m.arch).items()
    ]
    _bass_rust.insert_act_table_loads(self, tables)


def _build_nc():
    nc = bacc.Bacc("TRN2", target_bir_lowering=False, debug=False)
    nc.insert_act_table_loads = types.MethodType(_patched_insert_act_table_loads, nc)

    xs = nc.dram_tensor("xs", [ROWS, C], f32, kind="ExternalInput")
    # per-anchor clamped local flat pixel index (pos anchors then neg anchors)
    flat = nc.dram_tensor("flat", [NP + NN, 1], i32, kind="ExternalInput")
    # yz columns: [pos_y_local, pos_z, neg_y_local, neg_z]
    yz = nc.dram_tensor("yz", [NP, 4], i32, kind="ExternalInput")
    # tgt columns: [v_target0, v_target1, o_target]
    tgt = nc.dram_tensor("tgt", [NP, 3], f32, kind="ExternalInput")
    om = nc.dram_tensor("om", [NP, 1], u8, kind="ExternalInput")
    out = nc.dram_tensor("out", [1, 1], f32, kind="ExternalOutput")

    with tile.TileContext(nc) as tc:
        with (
            tc.tile_pool(name="sb", bufs=1) as pool,
            tc.tile_pool(name="ps", bufs=1, space="PSUM") as psum,
        ):
            # ======== critical path: offset DMA -> gather ========
            FL = pool.tile([128, 1], i32)
            nc.sync.dma_start(FL[:], flat[:, :])
            G = pool.tile([128, C], f32)
            nc.gpsimd.indirect_dma_start(
                out=G[:],
                out_offset=None,
                in_=xs[:],
                in_offset=bass.IndirectOffsetOnAxis(ap=FL[:, :1], axis=0),
            )
            # iota for the channel one-hot, second on the gpsimd stream
            IO = pool.tile([128, 5 * C], i32)
            nc.gpsimd.iota(IO[:], pattern=[[0, 5], [1, C]], base=0,
                           channel_multiplier=0)

            # ======== small inputs (overlap the gather flight) ========
            # one DMA for both halves: partitions 0-63 <- yz[:,0:2] (pos),
            # partitions 64-127 <- yz[:,2:4] (neg)
            IYZ = pool.tile([128, 2], i32)
            nc.sync.dma_start(IYZ[:], yz[:].rearrange("r (h c) -> h r c", h=2))
            yv = IYZ[:, 0:1]
            zv = IYZ[:, 1:2]

            TGT = pool.tile([128, 3], f32)
            nc.vector.memset(TGT[64:128, :], 0.0)
            nc.sync.dma_start(TGT[0:64, :], tgt[:, :])
            OM8 = pool.tile([128, 1], u8)
            nc.vector.memset(OM8[64:128, :], 0)
            nc.sync.dma_start(OM8[0:64, :], om[:, :])
            OMf = pool.tile([128, 1], f32)
            nc.vector.tensor_copy(OMf[:], OM8[:])

            # per-partition constants
            psign = pool.tile([128, 1], f32)   # +1 pos rows, -1 neg rows
            nc.vector.memset(psign[0:64, :], 1.0)
            nc.vector.memset(psign[64:128, :], -1.0)
            ispos = pool.tile([128, 1], f32)
            nc.vector.memset(ispos[0:64, :], 1.0)
            nc.vector.memset(ispos[64:128, :], 0.0)
            ones = pool.tile([128, 1], f32)
            nc.vector.memset(ones[:], 1.0)

            # n_o and its reciprocal, computed early (off the critical path)
            P1 = psum.tile([1, 1], f32)
            nc.tensor.matmul(out=P1[:], lhsT=ones[:], rhs=OMf[:], start=True, stop=True)
            rcp = pool.tile([1, 1], f32)
            nc.vector.reciprocal(rcp[:], P1[:])

            # validity: 0 <= y_local < HS  (single unsigned compare)
            vf = pool.tile([128, 1], f32)
            nc.vector.tensor_scalar(vf[:], yv.bitcast(u32), HS, None, op0=Alu.is_lt)
            vf128 = pool.tile([128, 1], f32)
            nc.vector.tensor_scalar(vf128[:], vf[:], 1.0 / (NP + NN), None, op0=Alu.mult)
            pv = pool.tile([128, 1], f32)
            nc.vector.tensor_tensor(pv[:], ispos[:], vf[:], op=Alu.mult)
            pvh = pool.tile([128, 1], f32)
            nc.vector.tensor_scalar(pvh[:], pv[:], 0.5, None, op0=Alu.mult)
            ompv = pool.tile([128, 1], f32)
            nc.vector.tensor_tensor(ompv[:], OMf[:], pv[:], op=Alu.mult)

            # channel targets (int32), order: [v0, v1, o, cls0, cls1]
            T = pool.tile([128, 5], i32)
            nc.vector.tensor_scalar(T[:, 0:1], zv, 2, None, op0=Alu.mult)
            nc.vector.tensor_scalar(T[:, 1:2], zv, 2, 1, op0=Alu.mult, op1=Alu.add)
            nc.vector.tensor_scalar(T[:, 2:3], zv, 1, 4 * K, op0=Alu.mult, op1=Alu.add)
            nc.vector.tensor_scalar(T[:, 3:4], zv, 2, 2 * K, op0=Alu.mult, op1=Alu.add)
            nc.vector.tensor_scalar(T[:, 4:5], zv, 2, 2 * K + 1, op0=Alu.mult, op1=Alu.add)

            IO3 = IO[:].rearrange("p (b c) -> p b c", c=C)
            T3 = T[:, :, None].to_broadcast([128, 5, C])
            MSK = pool.tile([128, 5 * C], f32)
            MSK3 = MSK[:].rearrange("p (b c) -> p b c", c=C)
            nc.vector.tensor_tensor(MSK3, IO3, T3, op=Alu.is_equal)

            # ======== post-gather: extract channels ========
            G3 = G[:, None, :].to_broadcast([128, 5, C])
            SEL = pool.tile([128, 5 * C], f32)
            SEL3 = SEL[:].rearrange("p (b c) -> p b c", c=C)
            nc.vector.tensor_tensor(SEL3, MSK3, G3, op=Alu.mult)
            E = pool.tile([128, 5], f32)
            nc.vector.reduce_sum(E[:, :, None], SEL3, axis=mybir.AxisListType.X)

            # ======== classification CE = ln(1 + exp(psign*(cls0-cls1))) =====
            D = pool.tile([128, 3], f32)
            dcls = pool.tile([128, 1], f32)
            nc.vector.tensor_tensor(dcls[:], E[:, 3:4], E[:, 4:5], op=Alu.subtract)
            ex = pool.tile([128, 1], f32)
            nc.scalar.activation(ex[:], dcls[:], Act.Exp, scale=psign[:])
            ce = pool.tile([128, 1], f32)
            nc.scalar.activation(ce[:], ex[:], Act.Ln, bias=1.0)
            nc.vector.tensor_tensor(D[:, 0:1], ce[:], vf128[:], op=Alu.mult)

            # ======== smooth-L1 on [v0-v0t, v1-v1t, o-ot] ========
            # sl1(d) = 0.5*min(d^2,1) + max(|d|,1) - 1
            dreg = pool.tile([128, 3], f32)
            nc.vector.tensor_tensor(dreg[:], E[:, 0:3], TGT[:, :], op=Alu.subtract)
            dsq = pool.tile([128, 3], f32)
            nc.vector.tensor_tensor(dsq[:], dreg[:], dreg[:], op=Alu.mult)
            mnqh = pool.tile([128, 3], f32)
            nc.vector.tensor_scalar(mnqh[:], dsq[:], 1.0, 0.5, op0=Alu.min, op1=Alu.mult)
            ngd = pool.tile([128, 3], f32)
            nc.vector.tensor_scalar(ngd[:], dreg[:], -1.0, None, op0=Alu.mult)
            av = pool.tile([128, 3], f32)
            nc.vector.tensor_tensor(av[:], dreg[:], ngd[:], op=Alu.max)
            mx1 = pool.tile([128, 3], f32)
            nc.vector.tensor_scalar(mx1[:], av[:], 1.0, 1.0, op0=Alu.max, op1=Alu.subtract)
            sl = pool.tile([128, 3], f32)
            nc.vector.tensor_tensor(sl[:], mnqh[:], mx1[:], op=Alu.add)

            lvs = pool.tile([128, 1], f32)
            nc.vector.tensor_tensor(lvs[:], sl[:, 0:1], sl[:, 1:2], op=Alu.add)
            nc.vector.tensor_tensor(D[:, 1:2], lvs[:], pvh[:], op=Alu.mult)
            nc.vector.tensor_tensor(D[:, 2:3], sl[:, 2:3], ompv[:], op=Alu.mult)

            # ======== partition reduction + combine ========
            P3 = psum.tile([1, 3], f32)
            nc.tensor.matmul(out=P3[:], lhsT=ones[:], rhs=D[:], start=True, stop=True)
            F = pool.tile([1, 3], f32)
            nc.vector.tensor_copy(F[:], P3[:])
            t = pool.tile([1, 1], f32)
            nc.vector.tensor_tensor(t[:], F[:, 1:2], F[:, 2:3], op=Alu.add)
            r = pool.tile([1, 1], f32)
            nc.vector.tensor_tensor(r[:], t[:], rcp[:], op=Alu.mult)
            res = pool.tile([1, 1], f32)
            nc.vector.tensor_tensor(res[:], F[:, 0:1], r[:], op=Alu.add)

            nc.sync.dma_start(out[:], res[:])

    nc.compile()
    return nc


def _get_nc():
    global _NC_CACHE
    if _NC_CACHE is None:
        _NC_CACHE = _build_nc()
    return _NC_CACHE


def make_in_maps(x, v_targets, o_targets, pos_y, pos_x, pos_z,
                 neg_y, neg_x, neg_z, o_mask):
    """Shard the full inputs into per-core input maps. Host work is limited to
    slicing x and translating anchor coordinates into each shard's local
    row-major layout; all math on x values happens on-device."""
    xr = np.ascontiguousarray(x).reshape(H * W, C)
    tgt = np.concatenate(
        [v_targets.astype(np.float32), o_targets.astype(np.float32)[:, None]], axis=1
    )
    omu = np.ascontiguousarray(o_mask).view(np.uint8).reshape(NP, 1)
    pos_y = pos_y.astype(np.int32)
    pos_x = pos_x.astype(np.int32)
    pos_z = pos_z.astype(np.int32)
    neg_y = neg_y.astype(np.int32)
    neg_x = neg_x.astype(np.int32)
    neg_z = neg_z.astype(np.int32)
    in_maps = []
    for i in range(NCORES):
        ply = pos_y - HS * i
        nly = neg_y - HS * i
        flat = np.concatenate(
            [
                np.clip(ply, 0, HS - 1) * W + pos_x,
                np.clip(nly, 0, HS - 1) * W + neg_x,
            ]
        ).astype(np.int32).reshape(NP + NN, 1)
        yzp = np.stack([ply, pos_z, nly, neg_z], axis=1).astype(np.int32)
        in_maps.append(
            {
                "xs": xr[HS * W * i: HS * W * (i + 1)],
                "flat": flat,
                "yz": yzp,
                "tgt": tgt,
                "om": omu,
            }
        )
    return in_maps


def kernel(**inputs):
    global LAST_RESULT
    nc = _get_nc()
    in_maps = make_in_maps(**inputs)
    res = run_bass_kernel_spmd(nc, in_maps, core_ids=list(range(NCORES)), trace=TRACE)
    LAST_RESULT = res
    total = np.float64(0.0)
    for core_out in res.results:
        total += np.float64(core_out["out"][0, 0])
    return np.array(np.float32(total))
